# revision 1
# baseline (speedup 1.0000x reference)
"""Trainium2 kernel for nn_Decoder_52664888983802.

est = einsum('bckE,wE->bckw', mixture_w, basis_weight); out = overlap_add(est, 8).

Sharding: batch dim (8) -> one batch row per NeuronCore (data parallel, no
collectives). Each core: mix [2, 16000, 512] f32 -> out [2, 128008] f32.
Best measured ~211 us on silicon (HBM roofline ~186 us; rel err ~2.4e-3).

G-phase strips (G=8 x15, then G=4, G=1 per c): a strip covers nf=128*G
frames with G frames per partition, DRAM-contiguous per partition (one
2*G KB load descriptor per partition; loads split in halves, SWDGE
f32->bf16 cast). PE transpose t yields ptr_t[e, j] = frame f0+G*j+t --
phase tiles aligned with the output grouping (row j holds 8*G consecutive
output samples -> 64*G-byte output DMA runs). DVE copies ptr->mxt (single
writer: est LDWEIGHTS has one sync-wait slot and ptr banks free on DVE
alone); ACT builds the B phase (frame f0-1+G*j, halo at j=0) from mxt,
off every critical chain. est phase matmuls (lhsT=mxt slice, rhs=wtT
chunk, N=16/8) accumulate slot-major in PSUM -- start=True clears
has_written for the whole bank, so each slot's group completes before the
next starts. DVE copies pest->esb, then G partition-aligned adds fold the
overlap-add: grp[j, 8t:8t+8] = est[f0+G*j+t, 0:8] + est[f0+G*j+t-1, 8:16].
The output side is traced one strip late (software pipelining), and
_prune_redundant_waits post-processes Tile's semaphores (transitively
implied waits dropped; several hw instruction structs accept a single
foreign sync wait).
"""

import math
import sys

sys.path.insert(0, "/opt/trn_rl_repo")

import numpy as np

import concourse.bass as bass
import concourse.mybir as mybir
import concourse.tile as tile
from concourse.bass_utils import run_bass_kernel_spmd

F32 = mybir.dt.float32
BF16 = mybir.dt.bfloat16

B, C, F, E, W = 8, 2, 16000, 512, 16
HALF = W // 2
OUTLEN = HALF * (F - 1) + W  # 128008
N_CORES = 8


def build_decoder(C=C, F=F, E=E, W=W):
    """G-phase strips: a strip covers nf = 128*G frames with G frames per
    partition, DRAM-contiguous per partition (one 2*G KB descriptor each).
    raw[p, t, e] = mix[f0 + G*p + t, e]; PE transpose t then yields
    ptr_t[e, j] = mix[f0 + G*j + t, e] -- phase tiles that align exactly
    with the output grouping: row j of the output group holds subframes
    f0 + G*j + t (t = 0..G-1), i.e. 8*G consecutive output samples.
    Per c: 15 strips of G=8, then G=4 and G=1 tails (15*1024+512+128=16000).
    """
    HALF = W // 2
    NCHUNK = E // 128
    OUTLEN = HALF * (F - 1) + W

    nc = bass.Bass()
    mix = nc.declare_dram_parameter("mixture_w", [C, F, E], F32, isOutput=False)
    wt = nc.declare_dram_parameter("wt", [128, NCHUNK, W], BF16, isOutput=False)
    id128 = nc.declare_dram_parameter("id128", [128, 128], BF16, isOutput=False)
    out = nc.declare_dram_parameter("out", [C, OUTLEN], F32, isOutput=True)

    GLIST = [8] * 15 + [4, 1]
    assert 128 * sum(GLIST) == F

    with tile.TileContext(nc) as tc:
        with (
            tc.tile_pool(name="consts", bufs=1) as consts,
            tc.tile_pool(name="rawp", bufs=4) as rawp,
            tc.tile_pool(name="mixtp", bufs=6) as mixtp,
            tc.tile_pool(name="mixtbp", bufs=6) as mixtbp,
            tc.tile_pool(name="estsbp", bufs=6) as estsbp,
            tc.tile_pool(name="quadp", bufs=6) as quadp,
            tc.tile_pool(name="ptransp", bufs=6, space="PSUM") as ptransp,
            tc.tile_pool(name="pestp", bufs=2, space="PSUM") as pestp,
        ):
            id128_sb = consts.tile([128, 128], BF16)
            nc.sync.dma_start(out=id128_sb[:], in_=id128[:])
            wt_sb = consts.tile([128, NCHUNK, W], BF16)
            nc.sync.dma_start(out=wt_sb[:], in_=wt[:])
            zero_sb = consts.tile([128, 1], BF16)
            nc.vector.memset(zero_sb[:], 0.0)

            # Warm-up PE ops: consume each const right after its DMA so that
            # steady-state PE instructions never need more than one
            # cross-engine wait.
            warm = ptransp.tile([128, 1024], BF16, tag="ptr", name="warm_t")
            nc.tensor.transpose(warm[:, 0:128], id128_sb[:], id128_sb[:])
            warm2 = pestp.tile([128, 9, W], F32, tag="est", name="warm_mm")
            nc.tensor.matmul(
                warm2[0:16, 0, :], lhsT=wt_sb[:, 0, :], rhs=wt_sb[:, 0, :]
            )

            prev_mxt = None
            prev_G = None
            pending = None

            def emit_back(c, s, f0, G, mxt, mxtb):
                # Output side of a strip, traced one strip late (software
                # pipelining): the scheduler then interleaves the next
                # strip's transposes with these MMs/copies, so PE never
                # stalls waiting for fresh PSUM->SBUF copies.
                nf = 128 * G
                # est matmuls, slot-major: start=True clears has_written for
                # the WHOLE bank, so each slot's 4-chunk accumulation group
                # must complete before the next slot starts.
                pest = pestp.tile(
                    [128, 9, W], F32, tag="est", name=f"pest_{c}_{s}"
                )
                for t in range(G):
                    for q in range(NCHUNK):
                        nc.tensor.matmul(
                            pest[:, t, :],
                            lhsT=mxt[:, q, t, :],
                            rhs=wt_sb[:, q, :],
                            start=q == 0, stop=q == NCHUNK - 1,
                        )
                for q in range(NCHUNK):
                    nc.tensor.matmul(
                        pest[:, G, HALF:W],
                        lhsT=mxtb[:, q, :],
                        rhs=wt_sb[:, q, HALF:W],
                        start=q == 0, stop=q == NCHUNK - 1,
                    )

                esb = estsbp.tile(
                    [128, 9, W], F32, tag="esb", name=f"esb_{c}_{s}"
                )
                nc.vector.tensor_copy(
                    out=esb[:, 0 : G + 1, :], in_=pest[:, 0 : G + 1, :]
                )

                grp = quadp.tile(
                    [128, 8, HALF], F32, tag="grp", name=f"grp_{c}_{s}"
                )
                for t in range(G):
                    pt = G if t == 0 else t - 1
                    nc.vector.tensor_add(
                        out=grp[:, t, :],
                        in0=esb[:, t, 0:HALF],
                        in1=esb[:, pt, HALF:W],
                    )

                with tc.high_priority(offset=-150):
                    nc.sync.dma_start(
                        out=out[
                            c, f0 * HALF : (f0 + nf) * HALF
                        ].rearrange("(p w) -> p w", p=128),
                        in_=grp[:, 0:G, :],
                    )
                nc.vector.memset(grp[0:1, 0:1, 0:1], 0.0)
                if G == 1:
                    # final subframe s=F: est[F-1, 8:16] = slot0 row 127
                    nc.sync.dma_start(
                        out=out[c, F * HALF : F * HALF + HALF].rearrange(
                            "(p w) -> p w", p=1
                        ),
                        in_=esb[127:128, 0, HALF:W],
                    )
                    nc.vector.memset(esb[96:128, 0, HALF : HALF + 1], 0.0)

            for c in range(C):
                f0 = 0
                for s, G in enumerate(GLIST):
                    nf = 128 * G
                    raw = rawp.tile(
                        [128, 8, E], BF16, tag="raw", name=f"raw_{c}_{s}"
                    )
                    dram = mix[c, f0 : f0 + nf, :].rearrange(
                        "(p t) e -> p t e", t=G
                    )
                    halves = [(0, G)] if G < 8 else [(0, 4), (4, 8)]
                    with tc.high_priority(offset=90):
                        for h0, h1 in halves:
                            nc.gpsimd.dma_start(
                                out=raw[:, h0:h1, :], in_=dram[:, h0:h1, :]
                            )

                    # mxt slots 0..G-1 = phase tiles (frame f0 + G*j + t),
                    # all DVE-written so ptr banks are freed by DVE alone.
                    # The B phase (frame f0-1+G*j; j=0 is the halo) lives in
                    # mxtb, built by ACT from mxt slot G-1 (SBUF->SBUF, a
                    # full strip of deferral slack, off every critical chain).
                    mxt = mixtp.tile(
                        [128, NCHUNK, 8, 128], BF16, tag="mixT",
                        name=f"mxt_{c}_{s}",
                    )
                    mxtb = mixtbp.tile(
                        [128, NCHUNK, 128], BF16, tag="mixB",
                        name=f"mxtb_{c}_{s}",
                    )
                    for q in range(NCHUNK):
                        ptr = ptransp.tile(
                            [128, 1024], BF16, tag="ptr", name=f"ptr_{c}_{s}_{q}"
                        )
                        for t in range(G):
                            nc.tensor.transpose(
                                ptr[:, t * 128 : (t + 1) * 128],
                                raw[:, t, q * 128 : (q + 1) * 128],
                                id128_sb[:],
                            )
                        # Main copy on DVE (single writer of mxt, so est
                        # LDWEIGHTS needs one wait and ptr banks are freed
                        # by DVE alone).
                        nc.vector.tensor_copy(
                            out=mxt[:, q, 0:G, :], in_=ptr[:, :nf]
                        )
                        # B phase on ACT, sourced from mxt (not ptr): B[j] =
                        # frame f0-1+G*j = mxt slot G-1 col j-1.
                        nc.scalar.copy(
                            out=mxtb[:, q, 1:128],
                            in_=mxt[:, q, G - 1, 0:127],
                        )
                        if s == 0:
                            nc.scalar.copy(
                                out=mxtb[:, q, 0:1], in_=zero_sb[:]
                            )
                        else:
                            nc.scalar.copy(
                                out=mxtb[:, q, 0:1],
                                in_=prev_mxt[:, q, prev_G - 1, 127:128],
                            )
                    prev_mxt, prev_G = mxt, G

                    if pending is not None:
                        emit_back(*pending)
                    pending = (c, s, f0, G, mxt, mxtb)
                    f0 += nf
            emit_back(*pending)
    _prune_redundant_waits(nc)
    return nc


def _prune_redundant_waits(nc):
    """Drop semaphore waits that are transitively guaranteed.

    Tile's add_semaphores is per-proc minimal but not transitively minimal,
    and several hardware instruction structs (the f32r self-loading
    LDWEIGHTS, HWDGE ring entries) have a single sync-wait slot, so extra
    waits fail walrus codegen ("Too many sync wait commands").

    Soundness: semaphores only increase during execution, and every
    dispatch unit (engine NX, HWDGE ring) executes wait-then-dispatch in
    program order. Hence (a) knowledge carried by the same proc's earlier
    instructions remains true, and (b) a wait (s >= v) is redundant if the
    producer instruction that raised s to v itself had knowledge implying
    it. Additionally, PE-self waits on Matmults are WAW guards for the
    64-deep LDWEIGHTS reorder window; actual MATMULs are strict-FIFO
    (pc-monotone start and end) and LDWEIGHTS only reads SBUF whose
    writers' waits are kept, so they are droppable when another wait
    remains."""
    insts = [i for blk in nc.m.functions[0].blocks for i in blk.instructions]

    # Monotonicity only holds for sems that are never decremented. Engine and
    # DMA sems only see sem-inc / positive sem-add-imm; the barrier_* sems
    # (preamble + kernel tail) use sem-dec/sem-sub and are left untouched.
    unsafe_sems = set()
    for inst in insts:
        si = inst.sync_info
        if si is None:
            continue
        for u in si.on_update or []:
            if u.sync_type != "semaphore":
                continue
            if u.update_mode not in ("sem-inc", "sem-add-imm") or (
                u.update_mode == "sem-add-imm" and u.update_value <= 0
            ):
                unsafe_sems.add(u.id)

    R = {}  # proc -> {sem_id: guaranteed value}
    sem_cum = {}  # sem_id -> cumulative update value
    producer_know = {}  # sem_id -> [(cum_value, knowledge)] in order

    def implied(w, know):
        return know.get(w.id, 0) >= w.wait_value

    def know_of_wait(w):
        k = {w.id: w.wait_value}
        for cv, pk in producer_know.get(w.id, []):
            if cv >= w.wait_value:
                for s2, v2 in pk.items():
                    k[s2] = max(k.get(s2, 0), v2)
                break
        return k

    for inst in insts:
        si = inst.sync_info
        if si is None:
            continue
        waits = list(si.on_wait or [])
        p = str(inst.engine)
        base = dict(R.get(p, {}))
        if any(
            w.sync_type != "semaphore"
            or w.wait_reg is not None
            or w.wait_mode != "sem-ge-imm"
            or w.id in unsafe_sems
            for w in waits
        ):
            kept = waits  # don't touch register/non-sem/barrier waits
        else:
            kept = []
            live = [w for w in waits if not implied(w, base)]
            # prefer a single wait whose producer knowledge implies the rest
            single = None
            for w in live:
                kw = dict(base)
                for s2, v2 in know_of_wait(w).items():
                    kw[s2] = max(kw.get(s2, 0), v2)
                if all(o is w or implied(o, kw) for o in live):
                    single = w
                    break
            if single is not None:
                kept = [single]
            else:
                # greedy: keep a wait only if not implied by base + kept so far
                for w in sorted(live, key=lambda w: -w.wait_value):
                    if not implied(w, base):
                        kept.append(w)
                        for s2, v2 in know_of_wait(w).items():
                            base[s2] = max(base.get(s2, 0), v2)
            if len(kept) > 1:
                # serial in-order engines: own-sem waits are satisfied by
                # the time the instruction executes (PE MATMULs are
                # pc-monotone; DVE/ACT are single-pipeline serial)
                own = {"PE": "PE_", "DVE": "DVE_", "Activation": "Activation_"}.get(
                    str(inst.engine).split(".")[-1]
                )
                if own is not None:
                    nonself = [w for w in kept if not w.ant_name.startswith(own)]
                    if nonself:
                        kept = nonself
            if len(kept) != len(waits):
                si.on_wait = kept
        # final knowledge for this inst (all original waits still held at
        # runtime even if pruned from the emitted instruction)
        know = dict(R.get(p, {}))
        for w in waits:
            if (
                w.sync_type == "semaphore"
                and w.wait_reg is None
                and w.wait_mode == "sem-ge-imm"
                and w.id not in unsafe_sems
            ):
                for s2, v2 in know_of_wait(w).items():
                    know[s2] = max(know.get(s2, 0), v2)
        R[p] = know
        for u in si.on_update or []:
            if u.sync_type != "semaphore" or u.id in unsafe_sems:
                continue
            sem_cum[u.id] = sem_cum.get(u.id, 0) + u.update_value
            producer_know.setdefault(u.id, []).append((sem_cum[u.id], dict(know)))


_NC_CACHE = {}


def _get_nc(**kw):
    key = tuple(sorted(kw.items()))
    if key not in _NC_CACHE:
        _NC_CACHE[key] = build_decoder(**kw)
    return _NC_CACHE[key]


def prep_aux_inputs(basis_weight):
    import ml_dtypes

    NCHUNK = E // 128
    # wt[p, q, w] = basis_weight[w, q*128 + p]  (wtT chunks, E on partitions)
    wtT = basis_weight.T.astype(np.float32)  # [E, W]
    wtc = wtT.reshape(NCHUNK, 128, W).transpose(1, 0, 2)  # [128, q, W]
    id128 = np.eye(128, dtype=np.float32)
    return (
        np.ascontiguousarray(wtc).astype(ml_dtypes.bfloat16),
        id128.astype(ml_dtypes.bfloat16),
    )


def kernel(mixture_w, basis_weight, _trace=False, **build_kw):
    mixture_w = np.ascontiguousarray(mixture_w, dtype=np.float32)
    basis_weight = np.ascontiguousarray(basis_weight, dtype=np.float32)
    assert mixture_w.shape == (B, C, F, E), mixture_w.shape
    assert basis_weight.shape == (W, E), basis_weight.shape

    nc = _get_nc(**build_kw)
    wt, id128 = prep_aux_inputs(basis_weight)
    in_maps = [
        {"mixture_w": mixture_w[i], "wt": wt, "id128": id128}
        for i in range(N_CORES)
    ]
    res = run_bass_kernel_spmd(
        nc, in_maps, core_ids=list(range(N_CORES)), trace=_trace
    )
    out = np.stack([res.results[i]["out"] for i in range(N_CORES)], axis=0)
    if _trace:
        kernel.last_exec_time_ns = res.exec_time_ns
        kernel.last_result = res
    return out

